# revision 38
# baseline (speedup 1.0000x reference)
"""Trainium2 Bass kernel for a dense transformer block (2x2048x1024, 16 heads,
MLP hidden 4096), SPMD over 8 NeuronCores.

Sharding: attention is head-sharded (2 heads per core, both batches); an
AllToAll converts head shards into token shards, after which proj/LN2/MLP run
on 512 tokens per core. All matmuls are bf16 with fp32 PSUM accumulation.

v2 restructure (from trace analysis of the 745us baseline):
- Attention software-pipelined: AV matmuls of block i interleave with the
  score matmuls of block i+1 so the PE never idles against the scalar
  engine's exp stream (the baseline held HAM at K=4/8 for 389us).
- exp issued as [128,1024] over a 2-bank PSUM scores tile (halves the
  352-cycle/instr ACT overhead).
- softmax 1/denom: broadcast the denominator via matmul FIRST, then one
  128-lane reciprocal_approx_fast (the baseline's [1,512] vector reciprocal
  was 3.3us each).
- LN chains use scalar-engine mu/mu^2 + reciprocal_approx_fast.
- LN2 is folded into the fc1 matmul epilogue (same affine fold as QKV), so
  the MLP starts right after the LN2 stats chain.
- residual hres computed from bf16 x during the A2A gap; v-transposes
  interleaved into phase 1; fc2/proj weights prefetched.
"""

from contextlib import ExitStack

import numpy as np
import ml_dtypes

import concourse.bass as bass
import concourse.mybir as mybir
from concourse import tile
from concourse.bass_utils import run_bass_kernel_spmd
from concourse.vector_clock import ScopedClock

F32 = mybir.dt.float32
BF16 = mybir.dt.bfloat16
AF = mybir.ActivationFunctionType
OP = mybir.AluOpType

N_CORES = 8
B, L, D = 2, 2048, 1024
NH, HD = 16, 64
HID = 4096
T = B * L            # 4096 tokens total
TOK = T // N_CORES   # 512 tokens per core after the A2A
KT = D // 128        # 8 k-tiles over the model dim
NJ = L // 128        # 16 j-tiles per batch
NI = L // TOK        # 4 i-blocks per batch
NB = B * NI          # 8 (b,i) attention blocks == 8 A2A destinations
EPS = 1e-6
GROUPS = [list(range(N_CORES))]

# bisect/tuning flags
FAST_RECIP = False   # custom-DVE reciprocal_approx_fast breaks this walrus build
LNEXP_RSQRT = True   # 1/std = exp(-0.5*ln(var+eps)) on scalar (no Sqrt, no table thrash)
MU_ON_SCALAR = True  # mu/mu^2 on the scalar engine vs vector
WIDE_EXP = True      # one [128,1024] exp per j vs two [128,512]


class ChunkedDrainTileContext(tile.TileContext):
    """This walrus build only accepts one explicit sem wait per CTRL
    instruction; split the kernel-tail drain's waits across a chain."""

    MAX_WAITS = 1

    def _drain_and_barrier(self, tick_clock, wait_clock):
        drain_inst = self.nc.sync.drain()
        wait_clock.add_sem_waits(
            drain_inst.ins, ScopedClock({None: tick_clock.global_clock})
        )
        si = drain_inst.ins.sync_info
        if si is not None and len(si.on_wait) > self.MAX_WAITS:
            waits = list(si.on_wait)
            si.on_wait = waits[: self.MAX_WAITS]
            for i in range(self.MAX_WAITS, len(waits), self.MAX_WAITS):
                extra = self.nc.sync.drain()
                extra.ins.sync_info = mybir.SyncInfo(
                    on_wait=waits[i : i + self.MAX_WAITS], on_update=[]
                )
        self.nc.all_engine_barrier()
        assert self.sems is not None
        popped = self.nc._tile_sem_poison_stack.pop()
        assert popped is self._sem_poison
        self.nc.clear_and_free_semaphores(list(self.sems.allocated().values()))
        self.nc.all_engine_barrier()


def _split_multi_waits(nc):
    """This walrus build accepts at most one sync wait per instruction; hoist
    extra waits onto preceding same-engine NoOps."""
    n = 0
    for fn in nc.m.functions:
        for bb in fn.blocks:
            insts = bb.instructions
            new = []
            for ins in insts:
                si = ins.sync_info
                if si is not None and len(si.on_wait) > 1:
                    waits = list(si.on_wait)
                    si.on_wait = [waits[-1]]
                    for w in waits[:-1]:
                        n += 1
                        nop = mybir.InstNoOp(
                            name=f"waitsplit-{n}",
                            sync_info=mybir.SyncInfo(on_wait=[w], on_update=[]),
                            bass_nofuse=True,
                            engine=ins.engine,
                        )
                        nc.register_instruction(nop)
                        new.append(nop)
                new.append(ins)
            if len(new) != len(insts):
                bb.instructions = new
    return n


def _ln_chain(nc, pool, psum_s, psum_q, eps_ap):
    """From replicated column sums / sums-of-squares, produce replicated
    A = 1/std and B = -mu/std tiles, all [128, TOK] f32.  mu and mu^2 run on
    the scalar engine; 1/std via the fast custom-DVE reciprocal."""
    inv_d = 1.0 / D
    mu = pool.tile([128, TOK], F32, tag="mu", name="mu", bufs=1)
    musq = pool.tile([128, TOK], F32, tag="musq", name="musq", bufs=1)
    if MU_ON_SCALAR:
        nc.scalar.mul(mu[:], psum_s[:], inv_d)
        nc.scalar.activation(musq[:], mu[:], AF.Square)
    else:
        nc.vector.tensor_scalar_mul(mu[:], psum_s[:], inv_d)
        nc.vector.scalar_tensor_tensor(
            musq[:], mu[:], 1.0, mu[:], OP.mult, OP.mult
        )
    var = pool.tile([128, TOK], F32, tag="var", name="var", bufs=1)
    nc.vector.scalar_tensor_tensor(
        var[:], psum_q[:], inv_d, musq[:], OP.mult, OP.subtract
    )
    a_t = pool.tile([128, TOK], F32, tag="a_t", name="a_t")
    if LNEXP_RSQRT:
        lv = pool.tile([128, TOK], F32, tag="lv", name="lv", bufs=1)
        nc.scalar.activation(lv[:], var[:], AF.Ln, bias=eps_ap)
        nc.scalar.activation(a_t[:], lv[:], AF.Exp, scale=-0.5)
    else:
        sv = pool.tile([128, TOK], F32, tag="sv", name="sv")
        nc.scalar.activation(sv[:], var[:], AF.Sqrt, bias=eps_ap)
        if FAST_RECIP:
            nc.vector.reciprocal_approx_fast(a_t[:], sv[:])
        else:
            nc.vector.reciprocal(a_t[:], sv[:])
    b_t = pool.tile([128, TOK], F32, tag="b_t", name="b_t")
    nc.vector.scalar_tensor_tensor(b_t[:], mu[:], -1.0, a_t[:], OP.mult, OP.mult)
    return a_t, b_t


def build_program():
    nc = bass.Bass(
        "TRN2", target_bir_lowering=False, debug=False, num_devices=N_CORES
    )

    xT = nc.dram_tensor("xT", [D, T], BF16, kind="ExternalInput")
    xres_bf = nc.dram_tensor("xres_bf", [D, TOK], BF16, kind="ExternalInput")
    wqT = nc.dram_tensor("wqT", [D, 128], BF16, kind="ExternalInput")
    wkT = nc.dram_tensor("wkT", [D, 128], BF16, kind="ExternalInput")
    wvT = nc.dram_tensor("wvT", [D, 128], BF16, kind="ExternalInput")
    wqs = nc.dram_tensor("wqs", [128, 1], F32, kind="ExternalInput")
    wks = nc.dram_tensor("wks", [128, 1], F32, kind="ExternalInput")
    wvs = nc.dram_tensor("wvs", [128, 1], F32, kind="ExternalInput")
    wpT = nc.dram_tensor("wpT", [D, D], BF16, kind="ExternalInput")
    fc1T = nc.dram_tensor("fc1T", [(HID // 128) * D, 128], BF16, kind="ExternalInput")
    fc2T = nc.dram_tensor("fc2T", [(D // 128) * HID, 128], BF16, kind="ExternalInput")
    w1rs = nc.dram_tensor("w1rs", [128, HID // 128], F32, kind="ExternalInput")
    g1c = nc.dram_tensor("g1c", [D, 1], F32, kind="ExternalInput")
    g2c = nc.dram_tensor("g2c", [D, 1], F32, kind="ExternalInput")
    ident = nc.dram_tensor("ident", [128, 128], BF16, kind="ExternalInput")
    outT = nc.dram_tensor("outT", [D, TOK], F32, kind="ExternalOutput")

    with ChunkedDrainTileContext(nc) as tc, ExitStack() as outer:
        p_const = outer.enter_context(tc.tile_pool(name="const", bufs=1))
        p_dram = outer.enter_context(tc.tile_pool(name="dram", bufs=1, space="DRAM"))
        # weights prefetched during attention (wpT, first fc1 tiles, xres)
        p_pref = outer.enter_context(tc.tile_pool(name="pref", bufs=1))

        ones = p_const.tile([128, 128], BF16, tag="ones", name="ones")
        nc.gpsimd.memset(ones[:], 1.0)
        idt = p_const.tile([128, 128], BF16, tag="idt", name="idt")
        nc.sync.dma_start(idt[:], ident[:])
        epst = p_const.tile([128, 1], F32, tag="epst", name="epst")
        nc.gpsimd.memset(epst[:], EPS)
        g1t = p_const.tile([128, KT], F32, tag="g1t", name="g1t")
        nc.sync.dma_start(
            g1t[:].rearrange("p (k c) -> p k c", k=KT),
            g1c[:].rearrange("(k p) c -> p k c", p=128),
        )
        g2t = p_const.tile([128, KT], F32, tag="g2t", name="g2t")
        nc.sync.dma_start(
            g2t[:].rearrange("p (k c) -> p k c", k=KT),
            g2c[:].rearrange("(k p) c -> p k c", p=128),
        )
        w1rst = p_const.tile([128, HID // 128], F32, tag="w1rst", name="w1rst")
        nc.sync.dma_start(w1rst[:], w1rs[:])

        # two half-A2As: dest j's tokens are [256j, 256j+256) within each
        # 2048-token half, so the first fires after attention blocks 0-3 and
        # hides behind blocks 4-7
        HTOK = TOK // 2
        sendA = p_dram.tile([T // 4, HTOK], BF16, tag="sendA", name="sendA")
        recvA = p_dram.tile([T // 4, HTOK], BF16, tag="recvA", name="recvA")
        sendB = p_dram.tile([T // 4, HTOK], BF16, tag="sendB", name="sendB")
        recvB = p_dram.tile([T // 4, HTOK], BF16, tag="recvB", name="recvB")

        with ExitStack() as attn_scope:
            p_keep = attn_scope.enter_context(tc.tile_pool(name="keep", bufs=1))
            qTt = p_keep.tile([128, T], BF16, tag="qT", name="qT")
            kTt = p_keep.tile([128, T], BF16, tag="kT", name="kT")
            vTt = p_keep.tile([128, T], BF16, tag="vT", name="vT")
            oh0 = p_keep.tile([64, T], BF16, tag="oh0", name="oh0")
            oh1 = p_keep.tile([64, T], BF16, tag="oh1", name="oh1")
            vones = [
                p_keep.tile([128, 130], BF16, tag=f"vo{t}", name=f"vo{t}")
                for t in range(T // 128)
            ]

            # ======== Phase 1: LN1 stats + QKV + v-transposes ========
            with (
                tc.tile_pool(name="xfp", bufs=1) as p_xf,
                tc.tile_pool(name="wqkv", bufs=1) as p_wqkv,
                tc.tile_pool(name="sq", bufs=4) as p_sq,
                tc.tile_pool(name="stats", bufs=2) as p_stats,
                tc.tile_pool(name="fix", bufs=2) as p_fix,
                tc.tile_pool(name="ps1", bufs=2, space="PSUM") as ps1,
                tc.tile_pool(name="ps1b", bufs=3, space="PSUM") as ps1b,
            ):
                # whole x rows as 8 contiguous 1MB DMAs (max DMA bandwidth,
                # no per-block pacing)
                xfull = []
                for k in range(KT):
                    t = p_xf.tile([128, T], BF16, tag=f"xf{k}", name=f"xf{k}")
                    nc.sync.dma_start(t[:], xT[128 * k : 128 * (k + 1), :])
                    xfull.append(t)
                wq, wk, wv = [], [], []
                for k in range(KT):
                    for lst, src, name in (
                        (wq, wqT, "wq"), (wk, wkT, "wk"), (wv, wvT, "wv")
                    ):
                        t = p_wqkv.tile([128, 128], BF16, tag=f"{name}{k}", name=f"{name}{k}")
                        nc.sync.dma_start(t[:], src[128 * k : 128 * (k + 1), :])
                        lst.append(t)
                wqsum = p_wqkv.tile([128, 1], F32, tag="wqsum", name="wqsum")
                nc.sync.dma_start(wqsum[:], wqs[:])
                wksum = p_wqkv.tile([128, 1], F32, tag="wksum", name="wksum")
                nc.sync.dma_start(wksum[:], wks[:])
                wvsum = p_wqkv.tile([128, 1], F32, tag="wvsum", name="wvsum")
                nc.sync.dma_start(wvsum[:], wvs[:])

                for tb in range(T // TOK):
                    sl = slice(TOK * tb, TOK * (tb + 1))
                    xtb = [xfull[k][:, sl] for k in range(KT)]
                    psum_s = ps1.tile([128, TOK], F32, tag="ps_s", name="ps_s")
                    for k in range(KT):
                        nc.tensor.matmul(
                            psum_s[:], ones[:], xtb[k][:],
                            start=(k == 0), stop=(k == KT - 1),
                        )
                    # QKV matmuls (epilogue needs the stats chain below)
                    pms = []
                    for w in (wq, wk, wv):
                        pm = ps1b.tile([128, TOK], F32, tag="ps_qkv", name="ps_qkv")
                        for k in range(KT):
                            nc.tensor.matmul(
                                pm[:], w[k][:], xtb[k][:],
                                start=(k == 0), stop=(k == KT - 1),
                            )
                        pms.append(pm)
                    # sum-of-squares stats (squares on scalar engine)
                    psum_q = ps1.tile([128, TOK], F32, tag="ps_q", name="ps_q")
                    for k in range(KT):
                        s = p_sq.tile([128, TOK], BF16, tag="sq", name="sq")
                        nc.scalar.activation(s[:], xtb[k][:], AF.Square)
                        nc.tensor.matmul(
                            psum_q[:], ones[:], s[:],
                            start=(k == 0), stop=(k == KT - 1),
                        )
                    a_t, b_t = _ln_chain(nc, p_stats, psum_s, psum_q, epst[:])
                    for dst, pm, wsum in (
                        (qTt, pms[0], wqsum), (kTt, pms[1], wksum), (vTt, pms[2], wvsum)
                    ):
                        u = p_fix.tile([128, TOK], F32, tag="fixu", name="fixu")
                        nc.vector.scalar_tensor_tensor(
                            u[:], pm[:], 1.0, a_t[:], OP.mult, OP.mult
                        )
                        nc.vector.scalar_tensor_tensor(
                            dst[:, sl], b_t[:], wsum[:], u[:], OP.mult, OP.add
                        )
                    # v-transposes for this block's four 128-token j-tiles
                    # (share the ps1b tag slots — 7 allocations/block, bufs=3)
                    for jj in range(4):
                        t128 = tb * 4 + jj
                        pv = ps1b.tile([128, 128], BF16, tag="ps_qkv", name="ps_vt")
                        nc.tensor.transpose(
                            pv[:], vTt[:, 128 * t128 : 128 * (t128 + 1)], idt[:]
                        )
                        vo = vones[t128]
                        nc.gpsimd.memset(vo[:], 1.0)
                        nc.vector.tensor_copy(vo[:, 0:64], pv[:, 0:64])
                        nc.vector.tensor_copy(vo[:, 65:129], pv[:, 64:128])

            # ======== Phase 2: attention, software-pipelined ========
            with (
                tc.tile_pool(name="exp", bufs=20) as p_exp,
                tc.tile_pool(name="attn", bufs=2) as p_attn,
                tc.tile_pool(name="pss", bufs=2, space="PSUM") as pss,
                tc.tile_pool(name="pso", bufs=2, space="PSUM") as pso,
            ):
                def block_bi(bi):
                    b, i = bi // NI, bi % NI
                    return b, i, slice(b * L + TOK * i, b * L + TOK * (i + 1))

                def finalize(bi, po0, po1, es_list):
                    b, i, isl = block_bi(bi)
                    # eagerly drain the AV accumulators to SBUF so the PSUM
                    # banks free up for the next block's AV matmuls
                    denb0 = p_attn.tile([1, TOK], BF16, tag="denb0", name="denb0")
                    nc.vector.tensor_copy(denb0[:], po0[64:65, :])
                    denb1 = p_attn.tile([1, TOK], BF16, tag="denb1", name="denb1")
                    nc.vector.tensor_copy(denb1[:], po1[64:65, :])
                    # both heads' AV into one tile, aligned with rri rows
                    ob = p_attn.tile([128, TOK], F32, tag="ob", name="ob")
                    nc.vector.tensor_copy(ob[0:64, :], po0[0:64, :])
                    nc.vector.tensor_copy(ob[64:128, :], po1[0:64, :])
                    prps = pss.tile([128, 2 * TOK], F32, tag="sc", name="prps")
                    nc.tensor.matmul(
                        prps[0:64, 0:TOK], ones[0:1, 0:64], denb0[:],
                        start=True, stop=True,
                    )
                    nc.tensor.matmul(
                        prps[64:128, 0:TOK], ones[0:1, 0:64], denb1[:],
                        start=True, stop=True,
                    )
                    rri = p_attn.tile([128, TOK], F32, tag="rri", name="rri")
                    nc.vector.reciprocal(rri[:], prps[:, 0:TOK])
                    nc.vector.scalar_tensor_tensor(
                        oh0[:, isl], ob[0:64, :], 1.0, rri[0:64, :],
                        OP.mult, OP.mult,
                    )
                    nc.vector.scalar_tensor_tensor(
                        oh1[:, isl], ob[64:128, :], 1.0, rri[64:128, :],
                        OP.mult, OP.mult,
                    )
                    # eager copies into the half-A2A send buffers: block bi
                    # covers dest chunks 2m and 2m+1 of its half (m = bi%4)
                    sbuf_, m = (sendA, bi) if bi < 4 else (sendB, bi - 4)
                    i0 = isl.start
                    for h, oht in ((0, oh0), (1, oh1)):
                        for c in range(2):
                            r = 128 * (2 * m + c) + 64 * h
                            nc.sync.dma_start(
                                sbuf_[r : r + 64, :],
                                oht[:, i0 + HTOK * c : i0 + HTOK * (c + 1)],
                            )

                wpt, w1pre, xrb = [], [], []
                prev = None  # (bi, es_list)
                for bi in range(NB + 1):
                    # prefetch DMAs on the otherwise-idle sync queue
                    if bi == 1:
                        for k in range(KT):
                            t = p_pref.tile([128, TOK], BF16, tag=f"xrb{k}", name=f"xrb{k}")
                            nc.sync.dma_start(t[:], xres_bf[128 * k : 128 * (k + 1), :])
                            xrb.append(t)
                    elif bi == 2:
                        for k in range(KT):
                            w = p_pref.tile([128, D], BF16, tag=f"wp{k}", name=f"wp{k}")
                            nc.sync.dma_start(w[:], wpT[128 * k : 128 * (k + 1), :])
                            wpt.append(w)
                    elif bi == 4:
                        for ht in range(3):
                            w1t = p_pref.tile([128, D], BF16, tag=f"w1p{ht}", name=f"w1p{ht}")
                            nc.sync.dma_start(
                                w1t[:].rearrange("p (k c) -> p k c", k=KT),
                                fc1T[D * ht : D * (ht + 1), :].rearrange(
                                    "(k p) c -> p k c", p=128
                                ),
                            )
                            w1pre.append(w1t)
                    if bi < NB:
                        b, i, isl = block_bi(bi)
                        es_list = []
                    ppo0 = ppo1 = None
                    for j in range(NJ):
                        # scores of this block first (they fill the PE while
                        # the previous block's finalize chain drains), then
                        # the AV matmuls of the previous block
                        if bi < NB:
                            jsl = slice(b * L + 128 * j, b * L + 128 * (j + 1))
                            sc = pss.tile([128, 2 * TOK], F32, tag="sc", name="sc")
                            nc.tensor.matmul(
                                sc[:, 0:TOK], kTt[0:64, jsl], qTt[0:64, isl],
                                start=True, stop=True, tile_position=(0, 0),
                            )
                            nc.tensor.matmul(
                                sc[:, TOK : 2 * TOK],
                                kTt[64:128, jsl], qTt[64:128, isl],
                                start=True, stop=True, tile_position=(64, 0),
                            )
                            es = p_exp.tile([128, 2 * TOK], BF16, tag="es", name="es")
                            if WIDE_EXP:
                                nc.scalar.activation(es[:], sc[:], AF.Exp)
                            else:
                                nc.scalar.activation(
                                    es[:, 0:TOK], sc[:, 0:TOK], AF.Exp
                                )
                                nc.scalar.activation(
                                    es[:, TOK : 2 * TOK], sc[:, TOK : 2 * TOK], AF.Exp
                                )
                            es_list.append(es)
                        if prev is not None:
                            pbi, pes = prev
                            pb = pbi // NI
                            if j == 0:
                                ppo0 = pso.tile([65, TOK], F32, tag="po0", name="po0")
                                ppo1 = pso.tile([65, TOK], F32, tag="po1", name="po1")
                            vo = vones[pb * NJ + j]
                            nc.tensor.matmul(
                                ppo0[:], vo[:, 0:65], pes[j][:, 0:TOK],
                                start=(j == 0), stop=(j == NJ - 1),
                            )
                            nc.tensor.matmul(
                                ppo1[:], vo[:, 65:130], pes[j][:, TOK : 2 * TOK],
                                start=(j == 0), stop=(j == NJ - 1),
                            )
                    if prev is not None:
                        finalize(prev[0], ppo0, ppo1, prev[1])
                        if prev[0] == 3:
                            # first half-A2A: fires mid-attention, hidden
                            nc.gpsimd.collective_compute(
                                "AllToAll", OP.bypass, replica_groups=GROUPS,
                                ins=[sendA[:].opt()], outs=[recvA[:].opt()],
                            )
                    if bi < NB:
                        prev = (bi, es_list)

        # ======== second half-A2A: head shards -> token shards ========
        nc.gpsimd.collective_compute(
            "AllToAll", OP.bypass, replica_groups=GROUPS,
            ins=[sendB[:].opt()], outs=[recvB[:].opt()],
        )

        # residual tiles live from here to the very end (opened after the
        # attention scope so they reuse its SBUF space)
        p_res = outer.enter_context(tc.tile_pool(name="res", bufs=1))

        # ======== hres (filler for the A2A gap): LN1(x_my)*g1 from bf16 x ==
        hres = [
            p_res.tile([128, TOK], F32, tag=f"hres{k}", name=f"hres{k}")
            for k in range(KT)
        ]
        with (
            tc.tile_pool(name="sqr", bufs=4) as p_sqr,
            tc.tile_pool(name="statr", bufs=1) as p_statr,
            tc.tile_pool(name="psr", bufs=1, space="PSUM") as psr,
        ):
            psum_s = psr.tile([128, TOK], F32, tag="ps_rs", name="ps_rs")
            for k in range(KT):
                nc.tensor.matmul(
                    psum_s[:], ones[:], xrb[k][:],
                    start=(k == 0), stop=(k == KT - 1),
                )
            psum_q = psr.tile([128, TOK], F32, tag="ps_rq", name="ps_rq")
            for k in range(KT):
                s = p_sqr.tile([128, TOK], BF16, tag="sqr", name="sqr")
                nc.scalar.activation(s[:], xrb[k][:], AF.Square)
                nc.tensor.matmul(
                    psum_q[:], ones[:], s[:],
                    start=(k == 0), stop=(k == KT - 1),
                )
            a_r, b_r = _ln_chain(nc, p_statr, psum_s, psum_q, epst[:])
            for k in range(KT):
                # hres = (x*a + b) * g1_col = x*(a*g1col) + b*g1col
                u = p_sqr.tile([128, TOK], F32, tag="resu", name="resu")
                nc.vector.scalar_tensor_tensor(
                    u[:], xrb[k][:], g1t[:, k : k + 1], a_r[:], OP.mult, OP.mult
                )
                nc.vector.scalar_tensor_tensor(
                    hres[k][:], b_r[:], g1t[:, k : k + 1], u[:], OP.mult, OP.add
                )

        # ======== Phase 3: proj + residual + LN2 stats (interleaved) ========
        hf_list, hrb_list = [], []
        with (
            tc.tile_pool(name="proj", bufs=1) as p_proj,
            tc.tile_pool(name="sq2", bufs=4) as p_sq2,
            tc.tile_pool(name="stats2", bufs=1) as p_stats2,
            tc.tile_pool(name="ps3", bufs=2, space="PSUM") as ps3,
            tc.tile_pool(name="ps3b", bufs=1, space="PSUM") as ps3b,
        ):
            ofull = []
            for k in range(KT):
                t = p_proj.tile([128, TOK], BF16, tag=f"of{k}", name=f"of{k}")
                nc.sync.dma_start(
                    t[:, 0:HTOK], recvA[128 * k : 128 * (k + 1), :]
                )
                nc.sync.dma_start(
                    t[:, HTOK:TOK], recvB[128 * k : 128 * (k + 1), :]
                )
                ofull.append(t)
            psum_s = ps3b.tile([128, TOK], F32, tag="ps_s2", name="ps_s2")
            psum_q = ps3b.tile([128, TOK], F32, tag="ps_q2", name="ps_q2")
            for dt in range(KT):
                pm = ps3.tile([128, TOK], F32, tag="ps_p", name="ps_p")
                for k in range(KT):
                    nc.tensor.matmul(
                        pm[:], wpt[k][:, 128 * dt : 128 * (dt + 1)], ofull[k][:],
                        start=(k == 0), stop=(k == KT - 1),
                    )
                hf = p_res.tile([128, TOK], F32, tag=f"hf{dt}", name=f"hf{dt}")
                nc.vector.scalar_tensor_tensor(
                    hf[:], pm[:], 1.0, hres[dt][:], OP.mult, OP.add
                )
                hf_list.append(hf)
                hb = p_res.tile([128, TOK], BF16, tag=f"hrb{dt}", name=f"hrb{dt}")
                nc.vector.tensor_copy(hb[:], hf[:])
                hrb_list.append(hb)
                nc.tensor.matmul(
                    psum_s[:], ones[:], hb[:],
                    start=(dt == 0), stop=(dt == KT - 1),
                )
                s = p_sq2.tile([128, TOK], BF16, tag="sq2", name="sq2")
                nc.scalar.activation(s[:], hb[:], AF.Square)
                nc.tensor.matmul(
                    psum_q[:], ones[:], s[:],
                    start=(dt == 0), stop=(dt == KT - 1),
                )
            a2, b2 = _ln_chain(nc, p_stats2, psum_s, psum_q, epst[:])
            # keep a2/b2 alive beyond this pool scope
            a2k = p_res.tile([128, TOK], F32, tag="a2k", name="a2k")
            nc.vector.tensor_copy(a2k[:], a2[:])
            b2k = p_res.tile([128, TOK], F32, tag="b2k", name="b2k")
            nc.vector.tensor_copy(b2k[:], b2[:])

        # ======== Phase 4: MLP with LN2 folded into the fc1 epilogue ========
        with (
            tc.tile_pool(name="m1p", bufs=1) as p_m1,
            tc.tile_pool(name="w1str", bufs=3) as p_w1,
            tc.tile_pool(name="w2str", bufs=2) as p_w2,
            tc.tile_pool(name="mlptmp", bufs=4) as p_t4,
            tc.tile_pool(name="out4", bufs=3) as p_out4,
            tc.tile_pool(name="ps4", bufs=2, space="PSUM") as ps4,
            tc.tile_pool(name="ps4b", bufs=2, space="PSUM") as ps4b,
        ):
            # fc2 weight tiles are prefetched mid-fc1 (after the first w1
            # tiles are in flight) so they don't block the fc1 stream
            w2tiles = []
            m1 = []
            for ht in range(HID // 128):
                if ht in (8, 20):
                    w2t = p_w2.tile([128, HID], BF16, tag="w2", name="w2")
                    nc.sync.dma_start(
                        w2t[:].rearrange("p (k c) -> p k c", k=HID // 128),
                        fc2T[HID * len(w2tiles) : HID * (len(w2tiles) + 1), :].rearrange(
                            "(k p) c -> p k c", p=128
                        ),
                    )
                    w2tiles.append(w2t)
                if ht < 3:
                    w1t = w1pre[ht]
                else:
                    w1t = p_w1.tile([128, D], BF16, tag="w1", name="w1")
                    nc.sync.dma_start(
                        w1t[:].rearrange("p (k c) -> p k c", k=KT),
                        fc1T[D * ht : D * (ht + 1), :].rearrange(
                            "(k p) c -> p k c", p=128
                        ),
                    )
                pm = ps4.tile([128, TOK], F32, tag="ps_m1", name="ps_m1")
                for k in range(KT):
                    nc.tensor.matmul(
                        pm[:], w1t[:, 128 * k : 128 * (k + 1)], hrb_list[k][:],
                        start=(k == 0), stop=(k == KT - 1),
                    )
                # LN2 fold: pre = pm*a2 + w1rowsum[ht]*b2
                u = p_t4.tile([128, TOK], F32, tag="m1u", name="m1u")
                nc.vector.scalar_tensor_tensor(
                    u[:], pm[:], 1.0, a2k[:], OP.mult, OP.mult
                )
                pre = p_t4.tile([128, TOK], F32, tag="m1pre", name="m1pre")
                nc.vector.scalar_tensor_tensor(
                    pre[:], b2k[:], w1rst[:, ht : ht + 1], u[:], OP.mult, OP.add
                )
                m = p_m1.tile([128, TOK], BF16, tag=f"m1_{ht}", name=f"m1_{ht}")
                nc.scalar.activation(m[:], pre[:], AF.Gelu)
                m1.append(m)
            # h2-pre tiles for the final residual: t2 = hf*a2 + b2  (g2 applied
            # in the fc2 epilogue)
            hgpre = []
            for k in range(KT):
                t1 = p_t4.tile([128, TOK], F32, tag="hg1", name="hg1")
                nc.vector.scalar_tensor_tensor(
                    t1[:], hf_list[k][:], 1.0, a2k[:], OP.mult, OP.mult
                )
                t2 = p_res.tile([128, TOK], F32, tag=f"hg{k}", name=f"hg{k}")
                nc.vector.scalar_tensor_tensor(
                    t2[:], b2k[:], 1.0, t1[:], OP.mult, OP.add
                )
                hgpre.append(t2)
            for dt in range(KT):
                if dt < 2:
                    w2t = w2tiles[dt]
                else:
                    w2t = p_w2.tile([128, HID], BF16, tag="w2", name="w2")
                    nc.sync.dma_start(
                        w2t[:].rearrange("p (k c) -> p k c", k=HID // 128),
                        fc2T[HID * dt : HID * (dt + 1), :].rearrange(
                            "(k p) c -> p k c", p=128
                        ),
                    )
                pm = ps4b.tile([128, TOK], F32, tag="ps_f2", name="ps_f2")
                for ht in range(HID // 128):
                    nc.tensor.matmul(
                        pm[:], w2t[:, 128 * ht : 128 * (ht + 1)], m1[ht][:],
                        start=(ht == 0), stop=(ht == HID // 128 - 1),
                    )
                # out = fc2 + (hf*a2 + b2)*g2
                ot = p_out4.tile([128, TOK], F32, tag="otile", name="otile")
                nc.vector.scalar_tensor_tensor(
                    ot[:], hgpre[dt][:], g2t[:, dt : dt + 1], pm[:],
                    OP.mult, OP.add,
                )
                nc.sync.dma_start(outT[128 * dt : 128 * (dt + 1), :], ot[:])

    _split_multi_waits(nc)
    return nc


_CACHED_NC = None


def _get_program():
    global _CACHED_NC
    if _CACHED_NC is None:
        _CACHED_NC = build_program()
    return _CACHED_NC


def _prepare_in_maps(x, w_qkv, w_proj, w_fc1, w_fc2, g1, g2):
    bf = ml_dtypes.bfloat16
    x2 = np.ascontiguousarray(np.asarray(x, np.float32).reshape(T, D))
    xT_b = np.ascontiguousarray(x2.T).astype(bf)

    g1 = np.asarray(g1, np.float32)
    g2 = np.asarray(g2, np.float32)
    wqkv_g = np.asarray(w_qkv, np.float32) * g1[None, :]
    scale = HD ** -0.5
    wpT_b = np.ascontiguousarray(np.asarray(w_proj, np.float32).T).astype(bf)
    fc1g = np.asarray(w_fc1, np.float32) * g2[None, :]
    fc1T_b = np.ascontiguousarray(
        fc1g.T.reshape(D, HID // 128, 128).transpose(1, 0, 2)
    ).astype(bf).reshape((HID // 128) * D, 128)
    fc2T_b = np.ascontiguousarray(
        np.asarray(w_fc2, np.float32).T.reshape(HID, D // 128, 128).transpose(1, 0, 2)
    ).astype(bf).reshape((D // 128) * HID, 128)
    w1rs_f = fc1g.astype(bf).astype(np.float32).sum(1)  # [HID]
    w1rs_b = np.ascontiguousarray(w1rs_f.reshape(HID // 128, 128).T)  # [128, 32]
    ident = np.eye(128, dtype=np.float32).astype(bf)
    g1c = np.ascontiguousarray(g1.reshape(D, 1))
    g2c = np.ascontiguousarray(g2.reshape(D, 1))

    def rowsum_bf(w):
        return np.ascontiguousarray(
            w.astype(bf).astype(np.float32).sum(1).reshape(128, 1)
        )

    in_maps = []
    for c in range(N_CORES):
        rows = slice(128 * c, 128 * (c + 1))
        wq_c = wqkv_g[rows, :] * scale            # scale folded into q
        wk_c = wqkv_g[D : 2 * D][rows, :]
        wv_c = wqkv_g[2 * D :][rows, :]
        # core c owns tokens [256c,256c+256) of each 2048-token half
        HT = TOK // 2
        xres_c = np.ascontiguousarray(
            np.concatenate(
                [x2[HT * c : HT * (c + 1)], x2[T // 2 + HT * c : T // 2 + HT * (c + 1)]]
            ).T
        )
        in_maps.append({
            "xT": xT_b,
            "xres_bf": xres_c.astype(bf),
            "wqT": np.ascontiguousarray(wq_c.T).astype(bf),
            "wkT": np.ascontiguousarray(wk_c.T).astype(bf),
            "wvT": np.ascontiguousarray(wv_c.T).astype(bf),
            "wqs": rowsum_bf(wq_c),
            "wks": rowsum_bf(wk_c),
            "wvs": rowsum_bf(wv_c),
            "wpT": wpT_b,
            "fc1T": fc1T_b,
            "fc2T": fc2T_b,
            "w1rs": w1rs_b,
            "g1c": g1c,
            "g2c": g2c,
            "ident": ident,
        })
    return in_maps


def run(inputs, trace=False, tmpdir=None):
    nc = _get_program()
    in_maps = _prepare_in_maps(**inputs)
    res = run_bass_kernel_spmd(
        nc, in_maps, list(range(N_CORES)), trace=trace, tmpdir=tmpdir
    )
    out = np.empty((T, D), np.float32)
    HT = TOK // 2
    for c in range(N_CORES):
        oT = res.results[c]["outT"]  # [D, 512]
        out[HT * c : HT * (c + 1), :] = oT[:, 0:HT].T
        out[T // 2 + HT * c : T // 2 + HT * (c + 1), :] = oT[:, HT:].T
    return out.reshape(B, L, D), res


def kernel(**inputs):
    out, _ = run(inputs, trace=False)
    return out
